# revision 33
# baseline (speedup 1.0000x reference)
"""Trainium2 Bass kernel for nn_Attention_16071767621814 (v2).

MobileViT-style attention block: 3x (depthwise3x3 conv + BN + 1x1 pointwise)
for q/k/v, 8-head attention (Lq=1024, Lkv=256, d=64), head-mixing reshape,
1x1 output projection.

Sharding: pure data-parallel over batch (16 batches / 8 cores = 2 per core),
zero collectives.

v2 design (vs v1 baseline at ~215us):
- depthwise convs moved off VectorE onto the PE as 9 diagonal-matmul taps
  accumulating in PSUM (exact fp32 accumulation; x cast once to bf16 into a
  zero-padded 36x36 grid; tap = strided window AP into that grid).
- pointwise q/k convs emit M=128-packed outputs (2 heads per PSUM tile):
  half the matmul count of v1.
- o_proj restructured: av is regathered (GpSimd copies) from per-pair
  [2*64d, (m,j)] layout into standard [inner-channel, out-token] layout,
  turning o_proj into 16 N=512 matmuls (vs 128 N=128).
- x is DMA'd as flat 4KB partition rows (v1 DMA'd 12k tiny descriptors into
  a padded image); padding/cast done on GpSimd.
- BN folded host-side exactly as v1 (scale into dw weights, bias via the
  pointwise const row driven by a ones-row in the rhs).
- softmax: exp on ScalarE (|S|/8 < 1 so no max-subtract), denominator via
  ones-lhsT matmul broadcast over 64 partitions, reciprocal_approx_fast +
  tensor_mul on VectorE.
"""

import numpy as np
import ml_dtypes

BF16NP = ml_dtypes.bfloat16

from concourse import bass, bacc, tile, mybir
from concourse.bass_utils import run_bass_kernel_spmd

F32 = mybir.dt.float32
BF16 = mybir.dt.bfloat16
AF = mybir.ActivationFunctionType
OP = mybir.AluOpType

NCORES = 8
B, C, S = 16, 192, 32
BPC = B // NCORES          # 2 batches per core
Lq = S * S                 # 1024
Sk = S // 2                # 16
Lkv = Sk * Sk              # 256
HEADS, HID, INNER = 8, 64, 512
EPS = 1e-5
G = 36                     # padded grid side (2 halo + 32 + 2)

TAPS = [(dy, dx) for dy in range(3) for dx in range(3)]

_NC = None
LAST_RESULT = None


def _build():
    nc = bacc.Bacc("TRN2", target_bir_lowering=False, debug=False,
                   num_devices=NCORES)

    x_ext = nc.declare_dram_parameter("x", [BPC, C, Lq], F32, isOutput=False)
    pw_ext, dgA_ext, dgB_ext = {}, {}, {}
    for p in ("q", "k", "v"):
        pw_ext[p] = nc.declare_dram_parameter(p + "pwT", [C + 1, INNER], BF16,
                                              isOutput=False)
        dgA_ext[p] = nc.declare_dram_parameter(p + "dgA", [128, 9, 128], BF16,
                                               isOutput=False)
        dgB_ext[p] = nc.declare_dram_parameter(p + "dgB", [64, 9, 64], BF16,
                                               isOutput=False)
    ows_ext = nc.declare_dram_parameter("owS", [128, 4, C], BF16,
                                        isOutput=False)
    ob_ext = nc.declare_dram_parameter("ob", [C, 1], F32, isOutput=False)
    out_ext = nc.declare_dram_parameter("out", [BPC, C, Lq], F32, isOutput=True)

    from contextlib import ExitStack
    with tile.TileContext(nc) as tc, ExitStack() as ctx:
        const = ctx.enter_context(tc.tile_pool(name="const", bufs=1))
        xpool = ctx.enter_context(tc.tile_pool(name="xpool", bufs=1))
        spool = ctx.enter_context(tc.tile_pool(name="spool", bufs=1))
        wpool = ctx.enter_context(tc.tile_pool(name="wpool", bufs=4))
        # ps1: ring of single-bank [<=128, <=512] fp32 tiles (4 banks)
        # pst: ring of two-bank [128, 1024] score tiles (4 banks)
        ps1 = ctx.enter_context(tc.tile_pool(name="ps1", bufs=4, space="PSUM"))
        pst = ctx.enter_context(tc.tile_pool(name="pst", bufs=2, space="PSUM"))

        # ---- weights to SBUF ----
        pwA, pwB, dgA, dgB = {}, {}, {}, {}
        for p in ("q", "k", "v"):
            pwA[p] = const.tile([128, INNER], BF16, name=f"pwA{p}")
            pwB[p] = const.tile([C + 1 - 128, INNER], BF16, name=f"pwB{p}")
            nc.sync.dma_start(out=pwA[p][:], in_=pw_ext[p][0:128, :])
            nc.sync.dma_start(out=pwB[p][:], in_=pw_ext[p][128:C + 1, :])
            dgA[p] = const.tile([128, 9, 128], BF16, name=f"dgA{p}")
            dgB[p] = const.tile([64, 9, 64], BF16, name=f"dgB{p}")
            nc.sync.dma_start(out=dgA[p][:], in_=dgA_ext[p][:])
            nc.sync.dma_start(out=dgB[p][:], in_=dgB_ext[p][:])
        owS = const.tile([128, 4, C], BF16, name="owS")
        nc.sync.dma_start(out=owS[:], in_=ows_ext[:])
        o_b0 = const.tile([128, 1], F32, name="ob0")
        o_b1 = const.tile([C - 128, 1], F32, name="ob1")
        nc.sync.dma_start(out=o_b0[:], in_=ob_ext[0:128, :])
        nc.sync.dma_start(out=o_b1[:], in_=ob_ext[128:C, :])
        ones64 = const.tile([128, HID], BF16, name="ones64")
        nc.vector.memset(ones64[:], 1.0)

        # ---- x: flat DMA + padded bf16 grid ----
        xfA = xpool.tile([128, BPC, Lq], F32, name="xfA")
        xfB = xpool.tile([64, BPC, Lq], F32, name="xfB")
        x36A = xpool.tile([128, BPC, G, G], BF16, name="x36A")
        x36B = xpool.tile([64, BPC, G, G], BF16, name="x36B")
        for x36 in (x36A, x36B):
            # zero only the halo borders; the interior is overwritten
            nc.vector.memset(x36[:, :, 0:2, :], 0.0)
            nc.vector.memset(x36[:, :, 34:36, :], 0.0)
            nc.vector.memset(x36[:, :, 2:34, 0:2], 0.0)
            nc.vector.memset(x36[:, :, 2:34, 34:36], 0.0)
        for bi in range(BPC):
            for lo, hi in ((0, 64), (64, 128)):
                nc.sync.dma_start(out=xfA[lo:hi, bi, :],
                                  in_=x_ext[bi, lo:hi, :])
            for lo, hi in ((128, 160), (160, 192)):
                nc.sync.dma_start(out=xfB[lo - 128:hi - 128, bi, :],
                                  in_=x_ext[bi, lo:hi, :])

        def pad_cast(bi):
            nc.vector.tensor_copy(
                x36A[:, bi, 2:2 + S, 2:2 + S],
                xfA[:, bi, :].rearrange("p (h w) -> p h w", h=S))
            nc.gpsimd.tensor_copy(
                x36B[:, bi, 2:2 + S, 2:2 + S],
                xfB[:, bi, :].rearrange("p (h w) -> p h w", h=S))

        # ---- dw conv outputs (bf16, ones row at partition 64 of B') ----
        xqbA = spool.tile([128, BPC, Lq], BF16, name="xqbA")
        xqbB = spool.tile([65, BPC, Lq], BF16, name="xqbB")
        xkbA = spool.tile([128, BPC, Lkv], BF16, name="xkbA")
        xkbB = spool.tile([65, BPC, Lkv], BF16, name="xkbB")
        xvbA = spool.tile([128, BPC, Lkv], BF16, name="xvbA")
        xvbB = spool.tile([65, BPC, Lkv], BF16, name="xvbB")
        nc.vector.memset(xqbB[64:65, :, :], 1.0)
        nc.vector.memset(xkbB[64:65, :, :], 1.0)
        nc.vector.memset(xvbB[64:65, :, :], 1.0)

        def dw_q(bi):
            # q: stride-1 taps, output natural (y,x) raster in PSUM.
            # Raster half n holds head-mix tokens i with i%128 in [64n,64n+64).
            for (x36, dg, np_, xqb) in ((x36A, dgA["q"], 128, xqbA),
                                        (x36B, dgB["q"], 64, xqbB)):
                for n in range(2):
                    qd = ps1.tile([np_, 512], F32, name="w1")
                    y0 = 16 * n
                    for t, (dy, dx) in enumerate(TAPS):
                        rhs = x36[:, bi, y0 + dy + 1:y0 + dy + 17,
                                  dx + 1:dx + 33]
                        nc.tensor.matmul(qd[:, :], dg[:, t, :], rhs,
                                         start=(t == 0), stop=(t == 8))
                    # cast + (j,m)->(m,j) token reorder for the head-mix trick
                    dst = xqb[0:np_, bi, :].rearrange(
                        "p (m j) -> p m j", m=8)[:, :, 64 * n:64 * n + 64]
                    nc.vector.tensor_copy(
                        dst, qd.rearrange("p (j m) -> p m j", m=8))

        def dw_kv(bi, p):
            # stride-2 taps
            xbA, xbB = (xkbA, xkbB) if p == "k" else (xvbA, xvbB)
            for (x36, dg, np_, xb) in ((x36A, dgA[p], 128, xbA),
                                       (x36B, dgB[p], 64, xbB)):
                kd = ps1.tile([np_, Lkv], F32, name="w1")
                for t, (dy, dx) in enumerate(TAPS):
                    nc.tensor.matmul(
                        kd[:, :], dg[:, t, :],
                        x36[:, bi, dy + 1:dy + 32:2, dx + 1:dx + 32:2],
                        start=(t == 0), stop=(t == 8))
                nc.vector.tensor_copy(xb[0:np_, bi, :], kd[:])

        # ---- pointwise: M=128-packed (2 heads per output chunk) ----
        def pw_block(bi):
            q_sb, k_sb = [], []
            for hp in range(4):
                hs = slice(hp * 128, (hp + 1) * 128)
                qs = wpool.tile([128, Lq], BF16, name=f"qsb{hp}", bufs=2)
                qp = pst.tile([128, Lq], F32, name="st")
                for n in range(2):
                    ns = slice(n * 512, (n + 1) * 512)
                    nc.tensor.matmul(qp[:, ns], pwA["q"][:, hs],
                                     xqbA[:, bi, ns], start=True, stop=False)
                    nc.tensor.matmul(qp[:, ns], pwB["q"][:, hs],
                                     xqbB[:, bi, ns], start=False, stop=True)
                nc.scalar.copy(qs[:], qp[:])
                q_sb.append(qs)

                kp = ps1.tile([128, Lkv], F32, name="w1")
                nc.tensor.matmul(kp[:], pwA["k"][:, hs], xkbA[:, bi, :],
                                 start=True, stop=False)
                nc.tensor.matmul(kp[:], pwB["k"][:, hs], xkbB[:, bi, :],
                                 start=False, stop=True)
                ks = wpool.tile([128, Lkv], BF16, name=f"ksb{hp}", bufs=2)
                nc.vector.tensor_copy(ks[:], kp[:])
                k_sb.append(ks)
            vT_sb = []
            for kc in range(2):
                kvs = slice(kc * 128, (kc + 1) * 128)
                vp = ps1.tile([128, INNER], F32, name="w1")
                nc.tensor.matmul(vp[:], xvbA[:, bi, kvs], pwA["v"][:],
                                 start=True, stop=False)
                nc.tensor.matmul(vp[:], xvbB[:, bi, kvs], pwB["v"][:],
                                 start=False, stop=True)
                vs = wpool.tile([128, INNER], BF16, name=f"vtsb{kc}", bufs=2)
                nc.scalar.copy(vs[:], vp[:])
                vT_sb.append(vs)
            return q_sb, k_sb, vT_sb

        avStds = {}

        def scores_part(bi, hp, q_sb, k_sb):
            h0, h1 = 2 * hp, 2 * hp + 1
            ex = {}
            # score matmuls interleaved across heads: h0 on PE rows 0-63,
            # h1 on rows 64-127 -> row-group concurrency. One exp ACT per
            # (h, kc) tile keeps the ScalarE instruction count low.
            for kc in range(2):
                kvs = slice(kc * 128, (kc + 1) * 128)
                stp = {h: pst.tile([128, Lq], F32, name="st")
                       for h in (h0, h1)}
                for n in range(2):
                    ns = slice(n * 512, (n + 1) * 512)
                    for h in (h0, h1):
                        io = (h % 2) * 64
                        nc.tensor.matmul(stp[h][:, ns],
                                         k_sb[hp][io:io + 64, kvs],
                                         q_sb[hp][io:io + 64, ns],
                                         start=True, stop=True)
                for h in (h0, h1):
                    e = wpool.tile([128, Lq], BF16, name="expA", bufs=6)
                    nc.scalar.activation(e[:], stp[h][:], AF.Exp,
                                         scale=1.0 / (HID ** 0.5))
                    ex[(h, kc)] = e
            return ex

        def av_part(bi, hp, ex, vT_sb):
            if hp == 0:
                avStds[bi] = wpool.tile([128, 4, Lq], BF16,
                                        name="avStd", bufs=2)
            avStd = avStds[bi]
            h0, h1 = 2 * hp, 2 * hp + 1
            # denominator input: kc-pair sum (bf16 2x DVE mode)
            exS = {}
            for h in (h0, h1):
                s = wpool.tile([128, Lq], BF16, name="exS", bufs=4)
                nc.vector.tensor_add(s[:], ex[(h, 0)][:], ex[(h, 1)][:])
                exS[h] = s

            # single-bank av/dn accumulators (n-split)
            avp = [ps1.tile([128, 512], F32, name="w1") for _ in range(2)]
            dnp = [ps1.tile([128, 512], F32, name="w1") for _ in range(2)]
            # av interleaved across heads on different banks: h0 streams the
            # n-bank while h1 streams the other -> col-group concurrency
            # without overlapping accumulation groups within a bank.
            for r in range(2):
                for kc in range(2):
                    for idx, h in enumerate((h0, h1)):
                        rows = slice(idx * 64, (idx + 1) * 64)
                        hs = slice(h * HID, (h + 1) * HID)
                        n = (idx + r) % 2
                        ns = slice(n * 512, (n + 1) * 512)
                        nc.tensor.matmul(avp[n][rows, :], vT_sb[kc][:, hs],
                                         ex[(h, kc)][:, ns],
                                         start=(kc == 0), stop=(kc == 1))
            for n in range(2):
                ns = slice(n * 512, (n + 1) * 512)
                for idx, h in enumerate((h0, h1)):
                    rows = slice(idx * 64, (idx + 1) * 64)
                    nc.tensor.matmul(dnp[n][rows, :], ones64[:, :],
                                     exS[h][:, ns], start=True, stop=True)
            rc = wpool.tile([128, Lq], F32, name="rcp", bufs=2)
            ab = wpool.tile([128, Lq], BF16, name="avbf", bufs=2)
            for n in range(2):
                ns = slice(n * 512, (n + 1) * 512)
                nc.vector.reciprocal_approx_fast(rc[:, ns], dnp[n][:])
                nc.vector.tensor_mul(ab[:, ns], avp[n][:], rc[:, ns])
            # regather into standard [inner-channel, out-token] layout
            # (bf16 SBUF->SBUF copies hit the DVE 4x perf mode)
            for idx, h in enumerate((h0, h1)):
                src = ab[idx * 64:(idx + 1) * 64, :].rearrange(
                    "p (m j) -> p m j", m=8)
                for par in range(2):
                    nc.vector.tensor_copy(
                        avStd[par * 64:(par + 1) * 64, :,
                              h * 128:(h + 1) * 128],
                        src[:, par::2, :])

        def oproj(bi):
            avStd = avStds[bi]
            op0 = [ps1.tile([128, 512], F32, name="w1") for _ in range(2)]
            op1 = [ps1.tile([C - 128, 512], F32, name="w1") for _ in range(2)]
            # ic-outer: consecutive matmuls share the stationary operand and
            # alternate PSUM banks (n), letting LDWEIGHTS overlap.
            for ic in range(4):
                for n in range(2):
                    ns = slice(n * 512, (n + 1) * 512)
                    nc.tensor.matmul(op0[n][:], owS[:, ic, 0:128],
                                     avStd[:, ic, ns],
                                     start=(ic == 0), stop=(ic == 3))
                for n in range(2):
                    ns = slice(n * 512, (n + 1) * 512)
                    nc.tensor.matmul(op1[n][:], owS[:, ic, 128:C],
                                     avStd[:, ic, ns],
                                     start=(ic == 0), stop=(ic == 3))
            for n in range(2):
                ns = slice(n * 512, (n + 1) * 512)
                os0 = wpool.tile([128, 512], F32, name="os0", bufs=2)
                os1 = wpool.tile([C - 128, 512], F32, name="os1", bufs=2)
                nc.scalar.activation(os0[:], op0[n][:], AF.Identity,
                                     bias=o_b0[:])
                nc.scalar.activation(os1[:], op1[n][:], AF.Identity,
                                     bias=o_b1[:])
                nc.sync.dma_start(out=out_ext[bi, 0:128, ns], in_=os0[:])
                nc.sync.dma_start(out=out_ext[bi, 128:C, ns], in_=os1[:])

        # ---- HAM warm-up: dense matmuls with no downstream consumers ----
        # (~4us of full-array activity while the x DMA lands, so the dw
        # phase starts at the 2.4 GHz clock instead of 1.2)
        for i in range(10):
            wu = ps1.tile([128, 512], F32, name="w1")
            nc.tensor.matmul(wu[:], pwA["q"][:, 0:128], pwA["v"][:],
                             start=True, stop=True)

        def attn_pair(bi, hp, q_sb, k_sb, vT_sb):
            ex = scores_part(bi, hp, q_sb, k_sb)
            av_part(bi, hp, ex, vT_sb)

        # ---- emission: software-pipelined across the 2 batches ----
        pad_cast(0)
        pad_cast(1)
        dw_q(0)
        dw_kv(0, "k")
        dw_kv(0, "v")
        t0 = pw_block(0)
        attn_pair(0, 0, *t0)
        dw_q(1)
        attn_pair(0, 1, *t0)
        dw_kv(1, "k")
        attn_pair(0, 2, *t0)
        dw_kv(1, "v")
        attn_pair(0, 3, *t0)
        oproj(0)
        t1 = pw_block(1)
        for hp in range(4):
            attn_pair(1, hp, *t1)
        oproj(1)

    nc.finalize()
    return nc


def _prep_weights(inputs):
    g = lambda k: np.asarray(inputs[k], np.float32)
    w = {}
    for p in ("q", "k", "v"):
        scale = g(p + "_bn_g") / np.sqrt(g(p + "_bn_v") + EPS)
        dww = g(p + "_dw")[:, 0].reshape(C, 9) * scale[:, None]
        biasc = g(p + "_bn_b") - g(p + "_bn_m") * scale
        pwm = g(p + "_pw")[:, :, 0, 0]
        const_row = pwm @ biasc
        w[p + "pwT"] = np.ascontiguousarray(
            np.concatenate([pwm.T, const_row[None, :]], 0)).astype(BF16NP)
        dgA = np.zeros((128, 9, 128), np.float32)
        dgA[np.arange(128), :, np.arange(128)] = dww[0:128, :]
        dgB = np.zeros((64, 9, 64), np.float32)
        dgB[np.arange(64), :, np.arange(64)] = dww[128:C, :]
        w[p + "dgA"] = dgA.astype(BF16NP)
        w[p + "dgB"] = dgB.astype(BF16NP)
    owt = g("o_w")[:, :, 0, 0].T  # [INNER, C]
    w["owS"] = np.ascontiguousarray(
        owt.reshape(4, 128, C).transpose(1, 0, 2)).astype(BF16NP)
    w["ob"] = np.ascontiguousarray(g("o_b")[:, None])
    return w


def kernel(**inputs):
    global _NC, LAST_RESULT
    if _NC is None:
        _NC = _build()
    w = _prep_weights(inputs)
    x = np.ascontiguousarray(
        np.asarray(inputs["x"], np.float32).reshape(B, C, Lq))
    in_maps = []
    for c in range(NCORES):
        m = {"x": np.ascontiguousarray(x[c * BPC:(c + 1) * BPC])}
        m.update(w)
        in_maps.append(m)
    res = run_bass_kernel_spmd(_NC, in_maps, list(range(NCORES)))
    LAST_RESULT = res
    out = np.concatenate([r["out"] for r in res.results], 0)
    return np.ascontiguousarray(out.reshape(B, C, S, S).astype(np.float32))


# revision 35
# speedup vs baseline: 1.0075x; 1.0075x over previous
"""Trainium2 Bass kernel for nn_Attention_16071767621814 (v2).

MobileViT-style attention block: 3x (depthwise3x3 conv + BN + 1x1 pointwise)
for q/k/v, 8-head attention (Lq=1024, Lkv=256, d=64), head-mixing reshape,
1x1 output projection.

Sharding: pure data-parallel over batch (16 batches / 8 cores = 2 per core),
zero collectives.

v2 design (vs v1 baseline at ~215us):
- depthwise convs moved off VectorE onto the PE as 9 diagonal-matmul taps
  accumulating in PSUM (exact fp32 accumulation; x cast once to bf16 into a
  zero-padded 36x36 grid; tap = strided window AP into that grid).
- pointwise q/k convs emit M=128-packed outputs (2 heads per PSUM tile):
  half the matmul count of v1.
- o_proj restructured: av is regathered (GpSimd copies) from per-pair
  [2*64d, (m,j)] layout into standard [inner-channel, out-token] layout,
  turning o_proj into 16 N=512 matmuls (vs 128 N=128).
- x is DMA'd as flat 4KB partition rows (v1 DMA'd 12k tiny descriptors into
  a padded image); padding/cast done on GpSimd.
- BN folded host-side exactly as v1 (scale into dw weights, bias via the
  pointwise const row driven by a ones-row in the rhs).
- softmax: exp on ScalarE (|S|/8 < 1 so no max-subtract), denominator via
  ones-lhsT matmul broadcast over 64 partitions, reciprocal_approx_fast +
  tensor_mul on VectorE.
"""

import numpy as np
import ml_dtypes

BF16NP = ml_dtypes.bfloat16

from concourse import bass, bacc, tile, mybir
from concourse.bass_utils import run_bass_kernel_spmd

F32 = mybir.dt.float32
BF16 = mybir.dt.bfloat16
AF = mybir.ActivationFunctionType
OP = mybir.AluOpType

NCORES = 8
B, C, S = 16, 192, 32
BPC = B // NCORES          # 2 batches per core
Lq = S * S                 # 1024
Sk = S // 2                # 16
Lkv = Sk * Sk              # 256
HEADS, HID, INNER = 8, 64, 512
EPS = 1e-5
G = 36                     # padded grid side (2 halo + 32 + 2)

TAPS = [(dy, dx) for dy in range(3) for dx in range(3)]

_NC = None
LAST_RESULT = None


def _build():
    nc = bacc.Bacc("TRN2", target_bir_lowering=False, debug=False,
                   num_devices=NCORES)

    x_ext = nc.declare_dram_parameter("x", [BPC, C, Lq], F32, isOutput=False)
    pw_ext, dgA_ext, dgB_ext = {}, {}, {}
    for p in ("q", "k", "v"):
        pw_ext[p] = nc.declare_dram_parameter(p + "pwT", [C + 1, INNER], BF16,
                                              isOutput=False)
        dgA_ext[p] = nc.declare_dram_parameter(p + "dgA", [128, 9, 128], BF16,
                                               isOutput=False)
        dgB_ext[p] = nc.declare_dram_parameter(p + "dgB", [64, 9, 64], BF16,
                                               isOutput=False)
    ows_ext = nc.declare_dram_parameter("owS", [128, 4, C], BF16,
                                        isOutput=False)
    ob_ext = nc.declare_dram_parameter("ob", [C, 1], F32, isOutput=False)
    out_ext = nc.declare_dram_parameter("out", [BPC, C, Lq], F32, isOutput=True)

    from contextlib import ExitStack
    with tile.TileContext(nc) as tc, ExitStack() as ctx:
        const = ctx.enter_context(tc.tile_pool(name="const", bufs=1))
        xpool = ctx.enter_context(tc.tile_pool(name="xpool", bufs=1))
        spool = ctx.enter_context(tc.tile_pool(name="spool", bufs=1))
        wpool = ctx.enter_context(tc.tile_pool(name="wpool", bufs=4))
        # ps1: ring of single-bank [<=128, <=512] fp32 tiles (4 banks)
        # pst: ring of two-bank [128, 1024] score tiles (4 banks)
        ps1 = ctx.enter_context(tc.tile_pool(name="ps1", bufs=4, space="PSUM"))
        pst = ctx.enter_context(tc.tile_pool(name="pst", bufs=2, space="PSUM"))

        # ---- weights to SBUF ----
        pwA, pwB, dgA, dgB = {}, {}, {}, {}
        for p in ("q", "k", "v"):
            pwA[p] = const.tile([128, INNER], BF16, name=f"pwA{p}")
            pwB[p] = const.tile([C + 1 - 128, INNER], BF16, name=f"pwB{p}")
            nc.sync.dma_start(out=pwA[p][:], in_=pw_ext[p][0:128, :])
            nc.sync.dma_start(out=pwB[p][:], in_=pw_ext[p][128:C + 1, :])
            dgA[p] = const.tile([128, 9, 128], BF16, name=f"dgA{p}")
            dgB[p] = const.tile([64, 9, 64], BF16, name=f"dgB{p}")
            nc.sync.dma_start(out=dgA[p][:], in_=dgA_ext[p][:])
            nc.sync.dma_start(out=dgB[p][:], in_=dgB_ext[p][:])
        owS = const.tile([128, 4, C], BF16, name="owS")
        nc.sync.dma_start(out=owS[:], in_=ows_ext[:])
        o_b0 = const.tile([128, 1], F32, name="ob0")
        o_b1 = const.tile([C - 128, 1], F32, name="ob1")
        nc.sync.dma_start(out=o_b0[:], in_=ob_ext[0:128, :])
        nc.sync.dma_start(out=o_b1[:], in_=ob_ext[128:C, :])
        ones64 = const.tile([128, HID], BF16, name="ones64")
        nc.vector.memset(ones64[:], 1.0)

        # ---- x: flat DMA + padded bf16 grid ----
        xfA = xpool.tile([128, BPC, Lq], F32, name="xfA")
        xfB = xpool.tile([64, BPC, Lq], F32, name="xfB")
        x36A = xpool.tile([128, BPC, G, G], BF16, name="x36A")
        x36B = xpool.tile([64, BPC, G, G], BF16, name="x36B")
        for x36 in (x36A, x36B):
            # zero only the halo borders; the interior is overwritten
            nc.vector.memset(x36[:, :, 0:2, :], 0.0)
            nc.vector.memset(x36[:, :, 34:36, :], 0.0)
            nc.vector.memset(x36[:, :, 2:34, 0:2], 0.0)
            nc.vector.memset(x36[:, :, 2:34, 34:36], 0.0)
        for bi in range(BPC):
            for lo, hi in ((0, 64), (64, 128)):
                nc.sync.dma_start(out=xfA[lo:hi, bi, :],
                                  in_=x_ext[bi, lo:hi, :])
            for lo, hi in ((128, 160), (160, 192)):
                nc.sync.dma_start(out=xfB[lo - 128:hi - 128, bi, :],
                                  in_=x_ext[bi, lo:hi, :])

        def pad_cast(bi):
            nc.vector.tensor_copy(
                x36A[:, bi, 2:2 + S, 2:2 + S],
                xfA[:, bi, :].rearrange("p (h w) -> p h w", h=S))
            nc.vector.tensor_copy(
                x36B[:, bi, 2:2 + S, 2:2 + S],
                xfB[:, bi, :].rearrange("p (h w) -> p h w", h=S))

        # ---- dw conv outputs (bf16, ones row at partition 64 of B') ----
        xqbA = spool.tile([128, BPC, Lq], BF16, name="xqbA")
        xqbB = spool.tile([65, BPC, Lq], BF16, name="xqbB")
        xkbA = spool.tile([128, BPC, Lkv], BF16, name="xkbA")
        xkbB = spool.tile([65, BPC, Lkv], BF16, name="xkbB")
        xvbA = spool.tile([128, BPC, Lkv], BF16, name="xvbA")
        xvbB = spool.tile([65, BPC, Lkv], BF16, name="xvbB")
        nc.vector.memset(xqbB[64:65, :, :], 1.0)
        nc.vector.memset(xkbB[64:65, :, :], 1.0)
        nc.vector.memset(xvbB[64:65, :, :], 1.0)

        def dw_q(bi):
            # q: stride-1 taps, output natural (y,x) raster in PSUM.
            # Raster half n holds head-mix tokens i with i%128 in [64n,64n+64).
            for (x36, dg, np_, xqb) in ((x36A, dgA["q"], 128, xqbA),
                                        (x36B, dgB["q"], 64, xqbB)):
                for n in range(2):
                    qd = ps1.tile([np_, 512], F32, name="w1")
                    y0 = 16 * n
                    for t, (dy, dx) in enumerate(TAPS):
                        rhs = x36[:, bi, y0 + dy + 1:y0 + dy + 17,
                                  dx + 1:dx + 33]
                        nc.tensor.matmul(qd[:, :], dg[:, t, :], rhs,
                                         start=(t == 0), stop=(t == 8))
                    # cast + (j,m)->(m,j) token reorder for the head-mix trick
                    dst = xqb[0:np_, bi, :].rearrange(
                        "p (m j) -> p m j", m=8)[:, :, 64 * n:64 * n + 64]
                    nc.vector.tensor_copy(
                        dst, qd.rearrange("p (j m) -> p m j", m=8))

        def dw_kv(bi, p):
            # stride-2 taps
            xbA, xbB = (xkbA, xkbB) if p == "k" else (xvbA, xvbB)
            for (x36, dg, np_, xb) in ((x36A, dgA[p], 128, xbA),
                                       (x36B, dgB[p], 64, xbB)):
                kd = ps1.tile([np_, Lkv], F32, name="w1")
                for t, (dy, dx) in enumerate(TAPS):
                    nc.tensor.matmul(
                        kd[:, :], dg[:, t, :],
                        x36[:, bi, dy + 1:dy + 32:2, dx + 1:dx + 32:2],
                        start=(t == 0), stop=(t == 8))
                nc.vector.tensor_copy(xb[0:np_, bi, :], kd[:])

        # ---- pointwise: M=128-packed (2 heads per output chunk) ----
        def pw_block(bi):
            q_sb, k_sb = [], []
            for hp in range(4):
                hs = slice(hp * 128, (hp + 1) * 128)
                qs = wpool.tile([128, Lq], BF16, name=f"qsb{hp}", bufs=2)
                qp = pst.tile([128, Lq], F32, name="st")
                for n in range(2):
                    ns = slice(n * 512, (n + 1) * 512)
                    nc.tensor.matmul(qp[:, ns], pwA["q"][:, hs],
                                     xqbA[:, bi, ns], start=True, stop=False)
                    nc.tensor.matmul(qp[:, ns], pwB["q"][:, hs],
                                     xqbB[:, bi, ns], start=False, stop=True)
                nc.scalar.copy(qs[:], qp[:])
                q_sb.append(qs)

                kp = ps1.tile([128, Lkv], F32, name="w1")
                nc.tensor.matmul(kp[:], pwA["k"][:, hs], xkbA[:, bi, :],
                                 start=True, stop=False)
                nc.tensor.matmul(kp[:], pwB["k"][:, hs], xkbB[:, bi, :],
                                 start=False, stop=True)
                ks = wpool.tile([128, Lkv], BF16, name=f"ksb{hp}", bufs=2)
                nc.vector.tensor_copy(ks[:], kp[:])
                k_sb.append(ks)
            vT_sb = []
            for kc in range(2):
                kvs = slice(kc * 128, (kc + 1) * 128)
                vp = ps1.tile([128, INNER], F32, name="w1")
                nc.tensor.matmul(vp[:], xvbA[:, bi, kvs], pwA["v"][:],
                                 start=True, stop=False)
                nc.tensor.matmul(vp[:], xvbB[:, bi, kvs], pwB["v"][:],
                                 start=False, stop=True)
                vs = wpool.tile([128, INNER], BF16, name=f"vtsb{kc}", bufs=2)
                nc.scalar.copy(vs[:], vp[:])
                vT_sb.append(vs)
            return q_sb, k_sb, vT_sb

        avStds = {}

        def scores_part(bi, hp, q_sb, k_sb):
            h0, h1 = 2 * hp, 2 * hp + 1
            ex = {}
            # score matmuls interleaved across heads: h0 on PE rows 0-63,
            # h1 on rows 64-127 -> row-group concurrency. One exp ACT per
            # (h, kc) tile keeps the ScalarE instruction count low.
            for kc in range(2):
                kvs = slice(kc * 128, (kc + 1) * 128)
                stp = {h: pst.tile([128, Lq], F32, name="st")
                       for h in (h0, h1)}
                for n in range(2):
                    ns = slice(n * 512, (n + 1) * 512)
                    for h in (h0, h1):
                        io = (h % 2) * 64
                        nc.tensor.matmul(stp[h][:, ns],
                                         k_sb[hp][io:io + 64, kvs],
                                         q_sb[hp][io:io + 64, ns],
                                         start=True, stop=True)
                for h in (h0, h1):
                    e = wpool.tile([128, Lq], BF16, name="expA", bufs=6)
                    nc.scalar.activation(e[:], stp[h][:], AF.Exp,
                                         scale=1.0 / (HID ** 0.5))
                    ex[(h, kc)] = e
            return ex

        def av_part(bi, hp, ex, vT_sb):
            if hp == 0:
                avStds[bi] = wpool.tile([128, 4, Lq], BF16,
                                        name="avStd", bufs=2)
            avStd = avStds[bi]
            h0, h1 = 2 * hp, 2 * hp + 1
            # denominator input: kc-pair sum (bf16 2x DVE mode)
            exS = {}
            for h in (h0, h1):
                s = wpool.tile([128, Lq], BF16, name="exS", bufs=4)
                nc.vector.tensor_add(s[:], ex[(h, 0)][:], ex[(h, 1)][:])
                exS[h] = s

            # single-bank av/dn accumulators (n-split)
            avp = [ps1.tile([128, 512], F32, name="w1") for _ in range(2)]
            dnp = [ps1.tile([128, 512], F32, name="w1") for _ in range(2)]
            # av interleaved across heads on different banks: h0 streams the
            # n-bank while h1 streams the other -> col-group concurrency
            # without overlapping accumulation groups within a bank.
            for r in range(2):
                for kc in range(2):
                    for idx, h in enumerate((h0, h1)):
                        rows = slice(idx * 64, (idx + 1) * 64)
                        hs = slice(h * HID, (h + 1) * HID)
                        n = (idx + r) % 2
                        ns = slice(n * 512, (n + 1) * 512)
                        nc.tensor.matmul(avp[n][rows, :], vT_sb[kc][:, hs],
                                         ex[(h, kc)][:, ns],
                                         start=(kc == 0), stop=(kc == 1))
            for n in range(2):
                ns = slice(n * 512, (n + 1) * 512)
                for idx, h in enumerate((h0, h1)):
                    rows = slice(idx * 64, (idx + 1) * 64)
                    nc.tensor.matmul(dnp[n][rows, :], ones64[:, :],
                                     exS[h][:, ns], start=True, stop=True)
            rc = wpool.tile([128, Lq], F32, name="rcp", bufs=2)
            ab = wpool.tile([128, Lq], BF16, name="avbf", bufs=2)
            for n in range(2):
                ns = slice(n * 512, (n + 1) * 512)
                nc.vector.reciprocal_approx_fast(rc[:, ns], dnp[n][:])
                nc.vector.tensor_mul(ab[:, ns], avp[n][:], rc[:, ns])
            # regather into standard [inner-channel, out-token] layout
            # (bf16 SBUF->SBUF copies hit the DVE 4x perf mode)
            for idx, h in enumerate((h0, h1)):
                src = ab[idx * 64:(idx + 1) * 64, :].rearrange(
                    "p (m j) -> p m j", m=8)
                for par in range(2):
                    nc.vector.tensor_copy(
                        avStd[par * 64:(par + 1) * 64, :,
                              h * 128:(h + 1) * 128],
                        src[:, par::2, :])

        def oproj(bi):
            avStd = avStds[bi]
            for n in range(2):
                ns = slice(n * 512, (n + 1) * 512)
                op0 = ps1.tile([128, 512], F32, name="w1")
                op1 = ps1.tile([C - 128, 512], F32, name="w1")
                for ic in range(4):
                    nc.tensor.matmul(op0[:], owS[:, ic, 0:128],
                                     avStd[:, ic, ns],
                                     start=(ic == 0), stop=(ic == 3))
                    nc.tensor.matmul(op1[:], owS[:, ic, 128:C],
                                     avStd[:, ic, ns],
                                     start=(ic == 0), stop=(ic == 3))
                os0 = wpool.tile([128, 512], F32, name="os0", bufs=2)
                os1 = wpool.tile([C - 128, 512], F32, name="os1", bufs=2)
                nc.scalar.activation(os0[:], op0[:], AF.Identity, bias=o_b0[:])
                nc.scalar.activation(os1[:], op1[:], AF.Identity, bias=o_b1[:])
                nc.sync.dma_start(out=out_ext[bi, 0:128, ns], in_=os0[:])
                nc.sync.dma_start(out=out_ext[bi, 128:C, ns], in_=os1[:])

        # ---- HAM warm-up: dense matmuls with no downstream consumers ----
        # (~4us of full-array activity while the x DMA lands, so the dw
        # phase starts at the 2.4 GHz clock instead of 1.2)
        for i in range(10):
            wu = ps1.tile([128, 512], F32, name="w1")
            nc.tensor.matmul(wu[:], pwA["q"][:, 0:128], pwA["v"][:],
                             start=True, stop=True)

        def attn_pair(bi, hp, q_sb, k_sb, vT_sb):
            ex = scores_part(bi, hp, q_sb, k_sb)
            av_part(bi, hp, ex, vT_sb)

        # ---- emission: software-pipelined across the 2 batches ----
        pad_cast(0)
        pad_cast(1)
        dw_q(0)
        dw_kv(0, "k")
        dw_kv(0, "v")
        t0 = pw_block(0)
        attn_pair(0, 0, *t0)
        dw_q(1)
        attn_pair(0, 1, *t0)
        dw_kv(1, "k")
        attn_pair(0, 2, *t0)
        dw_kv(1, "v")
        attn_pair(0, 3, *t0)
        oproj(0)
        t1 = pw_block(1)
        for hp in range(4):
            attn_pair(1, hp, *t1)
        oproj(1)

    nc.finalize()
    return nc


def _prep_weights(inputs):
    g = lambda k: np.asarray(inputs[k], np.float32)
    w = {}
    for p in ("q", "k", "v"):
        scale = g(p + "_bn_g") / np.sqrt(g(p + "_bn_v") + EPS)
        dww = g(p + "_dw")[:, 0].reshape(C, 9) * scale[:, None]
        biasc = g(p + "_bn_b") - g(p + "_bn_m") * scale
        pwm = g(p + "_pw")[:, :, 0, 0]
        const_row = pwm @ biasc
        w[p + "pwT"] = np.ascontiguousarray(
            np.concatenate([pwm.T, const_row[None, :]], 0)).astype(BF16NP)
        dgA = np.zeros((128, 9, 128), np.float32)
        dgA[np.arange(128), :, np.arange(128)] = dww[0:128, :]
        dgB = np.zeros((64, 9, 64), np.float32)
        dgB[np.arange(64), :, np.arange(64)] = dww[128:C, :]
        w[p + "dgA"] = dgA.astype(BF16NP)
        w[p + "dgB"] = dgB.astype(BF16NP)
    owt = g("o_w")[:, :, 0, 0].T  # [INNER, C]
    w["owS"] = np.ascontiguousarray(
        owt.reshape(4, 128, C).transpose(1, 0, 2)).astype(BF16NP)
    w["ob"] = np.ascontiguousarray(g("o_b")[:, None])
    return w


def kernel(**inputs):
    global _NC, LAST_RESULT
    if _NC is None:
        _NC = _build()
    w = _prep_weights(inputs)
    x = np.ascontiguousarray(
        np.asarray(inputs["x"], np.float32).reshape(B, C, Lq))
    in_maps = []
    for c in range(NCORES):
        m = {"x": np.ascontiguousarray(x[c * BPC:(c + 1) * BPC])}
        m.update(w)
        in_maps.append(m)
    res = run_bass_kernel_spmd(_NC, in_maps, list(range(NCORES)))
    LAST_RESULT = res
    out = np.concatenate([r["out"] for r in res.results], 0)
    return np.ascontiguousarray(out.reshape(B, C, S, S).astype(np.float32))
